# revision 32
# baseline (speedup 1.0000x reference)
"""CrossAttention Trainium2 Bass kernel (v2 — latency-optimized schedule).

Problem: B=2, Q=S=2048, D=1024, H=16 heads, A=64 head_dim.
  q = (iQ @ Wq)   -> [B,H,Q,A]
  k,v = iK @ Wkv  -> [B,H,S,A] each
  scores = q k^T / 8, mask -> -1e9, softmax over S
  out = (attn @ v) @ Wo -> [B,Q,D]

Sharding: 8 cores = 2 batches x 4 head-groups (4 heads each).
Each core computes a partial [Q, D] = ctx_local @ Wo_rows(local heads);
host sums the 4 partials per batch (row-parallel Wo unshard).

v2 changes vs the 193.8us baseline:
  - DMA issue order matches consumption: wk, ikt (s-chunk-major), wv,
    wq, mb, iqt qc0 (col-chunk-major), iqt qc1, wo.  kT/V/qT projections
    pipeline with the input stream; first matmul ~12us instead of ~18us,
    first exp ~28us instead of ~69us.
  - Attention emitted as one flat (qc,h,st) pipeline with the scores
    matmul one iteration ahead of the ctx matmul, so ACT exp runs
    back-to-back across head boundaries.
  - PE filler work (qT for the second q-chunk, Wo for the first) is
    interleaved INTO the attention st-loop in ~0.4-0.9us pieces to soak
    up the PE bubbles left when ACT paces.
  - Softmax denominator reciprocal via reciprocal_approx_fast (~5x
    faster than nc.vector.reciprocal; 18 bits is plenty ahead of a bf16
    multiply), so the norm chain fits inside a head period on DVE.
  - Output stored bf16 (halves the 8.4MB output DMA; host partial-sums
    in f32).
  - PSUM: scores 2x[128,1024] + ctx 1x + filler 1x = exactly 8 banks.
"""

import sys
import numpy as np

for _p in ("/opt/trn_rl_repo",):
    if _p not in sys.path:
        sys.path.insert(0, _p)

import ml_dtypes

B, Q, S, D = 2, 2048, 2048, 1024
H, A = 16, 64
HG = 4            # heads per core
NCORES = 8
NEG = -1e9
MIN_NST = 9       # S tiles after mask pruning (1152 slots; count ~1024)

_cache = {}


def _build_program(nst):
    import concourse.bass as bass  # noqa
    import concourse.bacc as bacc
    import concourse.tile as tile
    from concourse import mybir

    f32 = mybir.dt.float32
    bf16 = mybir.dt.bfloat16
    EXP = mybir.ActivationFunctionType.Exp
    MULT = mybir.AluOpType.mult

    nc = bacc.Bacc("TRN2", target_bir_lowering=False, debug=False)

    SP = nst * 128  # padded kept-S extent
    # DMA-friendly layouts: outer piece index keeps per-partition lines
    # long (6-8KB) so the single hardware DMA queue runs at full rate,
    # while pieces still land progressively for compute overlap.
    NKP = (nst + 2) // 3          # ikt pieces of 3 s-tiles (384 cols)
    KPW = 3 * 128
    iqt = nc.dram_tensor("iqt", [128, 4, 8, 512], bf16, kind="ExternalInput").ap()
    ikt = nc.dram_tensor("ikt", [128, NKP, 8, KPW], bf16, kind="ExternalInput").ap()
    wq = nc.dram_tensor("wq", [128, 8, 256], bf16, kind="ExternalInput").ap()
    wk = nc.dram_tensor("wk", [128, 8, 256], bf16, kind="ExternalInput").ap()
    wv = nc.dram_tensor("wv", [128, 8, 256], bf16, kind="ExternalInput").ap()
    wo = nc.dram_tensor("wo", [128, 2, D], bf16, kind="ExternalInput").ap()
    mb = nc.dram_tensor("mb", [128, nst], f32, kind="ExternalInput").ap()
    out = nc.dram_tensor("out", [128, 16, D], bf16, kind="ExternalOutput").ap()

    NDT = D // 128          # 8 d tiles

    with tile.TileContext(nc) as tc:
        with (
            tc.tile_pool(name="persist", bufs=1) as persist,
            tc.tile_pool(name="expp", bufs=4) as expp,
            tc.tile_pool(name="outp", bufs=3) as outp,
            tc.tile_pool(name="srp", bufs=2) as srp,
            tc.tile_pool(name="scp", bufs=2, space="PSUM") as scp,
            tc.tile_pool(name="ctxp", bufs=1, space="PSUM") as ctxp,
            tc.tile_pool(name="fillp", bufs=1, space="PSUM") as fillp,
        ):
            # ---- persistent tiles ----
            wk_sb = persist.tile([128, 8, 256], bf16, tag="wk")
            wq_sb = persist.tile([128, 8, 256], bf16, tag="wq")
            wv_sb = persist.tile([128, 8, 256], bf16, tag="wv")
            wo_sb = persist.tile([128, 2, D], bf16, tag="wo")
            iqt_sb = persist.tile([128, 4, 8, 512], bf16, tag="iqt")
            ikt_sb = persist.tile([128, NKP, 8, KPW], bf16, tag="ikt")
            mb_sb = persist.tile([128, nst], f32, tag="mb")
            qt_sb = persist.tile([128, 2, Q], bf16, tag="qt")
            kt_sb = persist.tile([128, 2, SP], bf16, tag="kt")
            # V padded to 128 cols; col 64 = ones (softmax denominator
            # row), cols 65.. = zeros.
            v_sb = persist.tile([128, nst, HG, 128], bf16, tag="v")
            ctxn = [
                [
                    persist.tile(
                        [128, 1024], bf16, tag=f"ctxn{qc}{t}", name=f"ctxn{qc}{t}"
                    )
                    for t in range(2)
                ]
                for qc in range(2)
            ]

            # ---- DMA issue order == consumption order ----
            nc.sync.dma_start(wk_sb[:], wk[:])
            for p in range(NKP):
                nc.sync.dma_start(ikt_sb[:, p], ikt[:, p])
            nc.sync.dma_start(wv_sb[:], wv[:])
            nc.sync.dma_start(wq_sb[:], wq[:])
            nc.sync.dma_start(mb_sb[:], mb[:])
            for p in range(4):
                nc.sync.dma_start(iqt_sb[:, p], iqt[:, p])
            nc.sync.dma_start(wo_sb[:], wo[:])

            # v ones/zeros columns (runs during the DMA-bound startup)
            nc.vector.memset(v_sb[:], 0.0)
            nc.vector.memset(v_sb[:, :, :, 64:65], 1.0)

            # ---- kT projection, piece-major (pipelines with ikt DMA) ----
            for p in range(NKP):
                for at in range(2):
                    ps = scp.tile([128, 1024], f32, tag="sc", name="ps")
                    for dt_i in range(NDT):
                        nc.tensor.matmul(
                            ps[:, :KPW],
                            lhsT=wk_sb[:, dt_i, at * 128:(at + 1) * 128],
                            rhs=ikt_sb[:, p, dt_i, :],
                            start=(dt_i == 0),
                            stop=(dt_i == NDT - 1),
                        )
                    nc.vector.tensor_copy(
                        out=kt_sb[:, at, p * KPW:(p + 1) * KPW], in_=ps[:, :KPW]
                    )

            # ---- V projection: natural [s, a] per head (+ ones col).
            # All of V runs pre-attention: it overlaps the iqt DMA that
            # gates the qT projection anyway, so deferring it into h0 only
            # crowds the PE there (measured +3us).
            NVPRE = nst

            def emit_v(st, pool, tag):
                p, j = divmod(st, 3)
                ps = pool.tile([128, HG, 64], f32, tag=tag, name="ps")
                for dt_i in range(NDT):
                    nc.tensor.matmul(
                        ps[:],
                        lhsT=ikt_sb[:, p, dt_i, j * 128:(j + 1) * 128],
                        rhs=wv_sb[:, dt_i, :],
                        start=(dt_i == 0),
                        stop=(dt_i == NDT - 1),
                    )
                nc.vector.tensor_copy(out=v_sb[:, st, :, 0:64], in_=ps[:])

            for st in range(NVPRE):
                emit_v(st, scp, "sc")

            # ---- qT projection, at=0 of q-chunk 0 only (all h0/h1 needs);
            #      at=1 runs as attention filler before h2 ----
            qt0 = scp.tile([128, 1024], f32, tag="sc", name="qt0")
            for p in (0, 1):
                for dt_i in range(NDT):
                    nc.tensor.matmul(
                        qt0[:, p * 512:(p + 1) * 512],
                        lhsT=wq_sb[:, dt_i, 0:128],
                        rhs=iqt_sb[:, p, dt_i, :],
                        start=(dt_i == 0),
                        stop=(dt_i == NDT - 1),
                    )
            nc.vector.tensor_copy(out=qt_sb[:, 0, 0:1024], in_=qt0[:])

            # ---- filler generators: PE work interleaved into attention ----
            def qt_fill_gen(at, qc):
                # one (at, qc) chunk of the q projection in 2-matmul pieces,
                # with the psum->sbuf copy attached to the last piece
                ps = fillp.tile([128, 1024], f32, tag="fill", name="ps")
                for pi, p in enumerate((2 * qc, 2 * qc + 1)):
                    for d0 in range(0, NDT, 2):
                        for dt_i in (d0, d0 + 1):
                            nc.tensor.matmul(
                                ps[:, pi * 512:(pi + 1) * 512],
                                lhsT=wq_sb[:, dt_i,
                                           at * 128:(at + 1) * 128],
                                rhs=iqt_sb[:, p, dt_i, :],
                                start=(dt_i == 0),
                                stop=(dt_i == NDT - 1),
                            )
                        last = (pi == 1 and d0 == NDT - 2)
                        if last:
                            nc.vector.tensor_copy(
                                out=qt_sb[:, at, qc * 1024:(qc + 1) * 1024],
                                in_=ps[:],
                            )
                        yield

            def emit_wo_mm(qt, ps, c):
                for t in range(2):
                    nc.tensor.matmul(
                        ps[:, c:c + 512],
                        lhsT=ctxn[qt // 8][t][:, (qt % 8) * 128:
                                              (qt % 8 + 1) * 128],
                        rhs=wo_sb[:, t, c:c + 512],
                        start=(t == 0),
                        stop=(t == 1),
                    )

            def emit_wo_out(qt, ps, on_act=False):
                ob = outp.tile([128, 1024], bf16, tag="ob", name="ob")
                if on_act:
                    nc.scalar.copy(out=ob[:], in_=ps[:])
                else:
                    nc.vector.tensor_copy(out=ob[:], in_=ps[:])
                nc.sync.dma_start(out[:, qt, :], ob[:])

            # qt 4-7 are reserved for the tail so the PE has ready work to
            # chew on while the last head's norm chain serializes (idling
            # there also drops the PE p-state, slowing the tail matmuls).
            def wo_qc0_gen():
                for qt in range(4):
                    ps = fillp.tile([128, 1024], f32, tag="fill", name="ps")
                    emit_wo_mm(qt, ps, 0)
                    yield
                    emit_wo_mm(qt, ps, 512)
                    emit_wo_out(qt, ps)
                    yield

            # ---- attention: flat (qc, h, st) pipeline, sc one ahead ----
            seq = [(qc, h, st) for qc in range(2) for h in range(HG)
                   for st in range(nst)]
            n = len(seq)

            def emit_sc(qc, h, st):
                po = (h % 2) * 64
                ti = h // 2
                ps = scp.tile([128, 1024], f32, tag="sc", name="ps")
                q0 = qc * 1024
                for c in (0, 512):
                    nc.tensor.matmul(
                        ps[:, c:c + 512],
                        lhsT=kt_sb[po:po + 64, ti, st * 128:(st + 1) * 128],
                        rhs=qt_sb[po:po + 64, ti, q0 + c:q0 + c + 512],
                        start=True,
                        stop=True,
                    )
                return ps

            def v_rest_gen():
                for st in range(NVPRE, nst):
                    emit_v(st, fillp, "fill")
                    yield

            # (filler, earliest (qc,h) it may be popped at): wo_qc0 reads
            # ctxn written by the qc0-h3 norm, so popping it earlier would
            # block the in-order PE queue on an unfired semaphore.
            fillers = [
                (v_rest_gen(), (0, 0)),        # V tiles 5.. feed h0's ctx
                (qt_fill_gen(1, 0), (0, 1)),   # at=1 of qc0, before h2
                (qt_fill_gen(0, 1), (0, 2)),
                (qt_fill_gen(1, 1), (0, 2)),
                (wo_qc0_gen(), (1, 1)),
            ]
            fill_i = 0  # index into fillers; advance when exhausted

            def pop_filler(pos):
                nonlocal fill_i
                while fill_i < len(fillers):
                    gen, gate = fillers[fill_i]
                    if pos is not None and pos < gate:
                        return False
                    try:
                        next(gen)
                        return True
                    except StopIteration:
                        fill_i += 1
                return False

            sc_tiles = {}
            sc_tiles[0] = emit_sc(*seq[0])
            ctx_cur = None
            for i, (qc, h, st) in enumerate(seq):
                # one-ahead scores matmul
                if i + 1 < n:
                    sc_tiles[i + 1] = emit_sc(*seq[i + 1])
                ps = sc_tiles.pop(i)
                ex = expp.tile([128, 1024], bf16, tag="exp", name="ex")
                nc.scalar.activation(
                    out=ex[:], in_=ps[:], func=EXP,
                    bias=mb_sb[:, st:st + 1], scale=0.125,
                )
                if st == 0:
                    ctx_cur = ctxp.tile([128, 1024], f32, tag="ctx", name="ctx")
                for c in (0, 512):
                    nc.tensor.matmul(
                        ctx_cur[:, c:c + 512],
                        lhsT=v_sb[:, st, h, :],
                        rhs=ex[:, c:c + 512],
                        start=(st == 0),
                        stop=(st == nst - 1),
                    )
                # one filler piece per iteration (gates in `fillers` keep
                # not-yet-satisfiable work out of the in-order PE queue)
                pop_filler((qc, h))
                if st == nst - 1:
                    po = (h % 2) * 64
                    ti = h // 2
                    # normalize: ctx/denom -> ctxn (packed 2 heads).
                    # reciprocal_approx_fast is a custom DVE op that only
                    # reads partition 0, so the denominator row must first
                    # be copied to its own partition-0 tile.
                    den = srp.tile([1, 1024], f32, tag="den", name="den")
                    ctxu = srp.tile([65, 1024], f32, tag="ctxu", name="ctxu")
                    recip = srp.tile([1, 1024], f32, tag="recip", name="recip")
                    bcd = srp.tile([64, 1024], f32, tag="bcd", name="bcd")
                    if i == n - 1:
                        # Last head (critical path): denominator copy on the
                        # idle ACT engine, and the recip/broadcast/multiply
                        # chain split into 512-col halves — the tail Wo for
                        # qt8-11 only reads cols 0-511 of ctxn, so it
                        # unblocks after the first half-multiply.
                        nc.scalar.copy(out=den[:], in_=ctx_cur[64:65, :])
                        nc.vector.tensor_copy(
                            out=ctxu[0:64, :], in_=ctx_cur[0:64, :]
                        )
                        for c in (0, 512):
                            nc.vector.reciprocal_approx_fast(
                                recip[:, c:c + 512], den[:, c:c + 512]
                            )
                        for c in (0, 512):
                            nc.gpsimd.partition_broadcast(
                                bcd[:, c:c + 512], recip[:, c:c + 512]
                            )
                            nc.vector.tensor_tensor(
                                ctxn[qc][ti][po:po + 64, c:c + 512],
                                ctxu[0:64, c:c + 512], bcd[:, c:c + 512], MULT,
                            )
                    else:
                        # One [65,1024] copy frees the ctx PSUM bank for the
                        # next head ~1us sooner; den is then sliced in SBUF.
                        nc.vector.tensor_copy(out=ctxu[:], in_=ctx_cur[:65, :])
                        nc.vector.tensor_copy(out=den[:], in_=ctxu[64:65, :])
                        nc.vector.reciprocal_approx_fast(recip[:], den[:])
                        nc.gpsimd.partition_broadcast(bcd[:], recip[:])
                        nc.vector.tensor_tensor(
                            ctxn[qc][ti][po:po + 64, :],
                            ctxu[0:64, :], bcd[:], MULT,
                        )

            # ---- drain leftover fillers, then tail Wo ----
            # qt 4-7 (q-chunk 0) first: they depend only on qc0 ctxn, so the
            # PE runs them while the last head's norm chain completes.
            # Three-slot psum rotation (scp x2 + fillp) and output copies
            # split across the now-idle ACT and DVE keep the pace at the
            # matmul rate instead of the copy rate.
            while pop_filler(None):
                pass
            for k, qt in enumerate(range(4, 16)):
                if k % 3 == 2:
                    ps = fillp.tile([128, 1024], f32, tag="fill", name="ps")
                else:
                    ps = scp.tile([128, 1024], f32, tag="sc", name="ps")
                emit_wo_mm(qt, ps, 0)
                emit_wo_mm(qt, ps, 512)
                ob = outp.tile([128, 1024], bf16, tag="ob", name="ob")
                nc.scalar.copy(out=ob[:, 0:512], in_=ps[:, 0:512])
                nc.vector.tensor_copy(out=ob[:, 512:1024], in_=ps[:, 512:1024])
                nc.sync.dma_start(out[:, qt, :], ob[:])

    nc.compile()
    return nc


def _get_program(nst):
    if nst not in _cache:
        _cache[nst] = _build_program(nst)
    return _cache[nst]


def _prep_inputs(iQ, iK, mask, Wq, Wkv, Wo):
    """Build the 8 per-core input maps (host-side shard + prune + cast)."""
    bf = ml_dtypes.bfloat16
    iQ = np.asarray(iQ, dtype=np.float32)
    iK = np.asarray(iK, dtype=np.float32)
    mask = np.asarray(mask)
    Wq = np.asarray(Wq, dtype=np.float32)
    Wkv = np.asarray(Wkv, dtype=np.float32)
    Wo = np.asarray(Wo, dtype=np.float32)

    def tile_kxn(a):  # [K=1024, N] -> [128, K/128, N]
        K, N = a.shape
        return np.ascontiguousarray(
            a.reshape(K // 128, 128, N).transpose(1, 0, 2)
        )

    kept = [np.flatnonzero(~mask[b, 0]) for b in range(B)]
    nst = max(MIN_NST, max((len(k) + 127) // 128 for k in kept))
    nst = ((nst + 2) // 3) * 3   # kernel wants 3-s-tile DMA pieces
    SP = nst * 128

    per_b = {}
    for b in range(B):
        nk = len(kept[b])
        ikt_full = np.zeros((1024, SP), dtype=np.float32)
        ikt_full[:, :nk] = iK[b][kept[b], :].T
        bias = np.full(SP, np.float32(NEG), dtype=np.float32)
        bias[:nk] = 0.0
        # [128, 8, SP] -> piece-major [128, SP/384, 8, 384] so each DMA
        # piece is one contiguous long per-partition line
        ikt_t = tile_kxn(ikt_full).astype(bf)
        ikt_t = np.ascontiguousarray(
            ikt_t.reshape(128, 8, nst // 3, 384).transpose(0, 2, 1, 3)
        )
        iqt_t = tile_kxn(iQ[b].T).astype(bf)
        iqt_t = np.ascontiguousarray(
            iqt_t.reshape(128, 8, 4, 512).transpose(0, 2, 1, 3)
        )
        per_b[b] = {
            "iqt": iqt_t,
            "ikt": ikt_t,
            "mb": np.ascontiguousarray(bias.reshape(nst, 128).T),
        }
    in_maps = []
    for c in range(NCORES):
        b, g = divmod(c, NCORES // B)
        cols = slice(g * 256, (g + 1) * 256)
        wo_g = Wo[g * 256:(g + 1) * 256, :]          # [256, 1024]
        in_maps.append({
            "iqt": per_b[b]["iqt"],
            "ikt": per_b[b]["ikt"],
            "mb": per_b[b]["mb"],
            "wq": tile_kxn(Wq[:, cols]).astype(bf),
            "wk": tile_kxn(Wkv[:, cols]).astype(bf),
            "wv": tile_kxn(Wkv[:, 1024 + g * 256:1024 + (g + 1) * 256]).astype(bf),
            "wo": np.ascontiguousarray(
                wo_g.reshape(2, 128, D).transpose(1, 0, 2)
            ).astype(bf),
        })
    return in_maps, nst


def _run(inputs, trace=False):
    from concourse.bass_utils import run_bass_kernel_spmd

    in_maps, nst = _prep_inputs(**inputs)
    nc = _get_program(nst)
    res = run_bass_kernel_spmd(
        nc, in_maps, list(range(NCORES)), trace=trace
    )
    outs = []
    for b in range(B):
        acc = None
        for g in range(NCORES // B):
            o = np.asarray(
                res.results[b * (NCORES // B) + g]["out"], dtype=np.float32
            )
            acc = o if acc is None else acc + o
        # [128, 16, 1024] -> [2048, 1024]
        outs.append(acc.transpose(1, 0, 2).reshape(Q, D))
    return np.stack(outs), res


def kernel(**inputs):
    out, _ = _run(inputs, trace=False)
    return out


# revision 33
# speedup vs baseline: 1.0123x; 1.0123x over previous
"""CrossAttention Trainium2 Bass kernel (v2 — latency-optimized schedule).

Problem: B=2, Q=S=2048, D=1024, H=16 heads, A=64 head_dim.
  q = (iQ @ Wq)   -> [B,H,Q,A]
  k,v = iK @ Wkv  -> [B,H,S,A] each
  scores = q k^T / 8, mask -> -1e9, softmax over S
  out = (attn @ v) @ Wo -> [B,Q,D]

Sharding: 8 cores = 2 batches x 4 head-groups (4 heads each).
Each core computes a partial [Q, D] = ctx_local @ Wo_rows(local heads);
host sums the 4 partials per batch (row-parallel Wo unshard).

Optimizations vs the 193.8us baseline (now ~154us cold):
  - DMA issue order matches consumption (wk, ikt, wv, wq, mb, iqt qc0,
    iqt qc1, wo) with piece-major host layouts so every piece is one
    long-line DMA (~345GB/s) that lands just before its projection runs;
    first exp fires at ~38us instead of ~69us.
  - Attention emitted as one flat (qc,h,st) pipeline with the scores
    matmul one iteration ahead of the ctx matmul; the exp stream on ACT
    then runs at its floor cadence (~1.28us per [128,1024] tile), which
    is the binding resource of the whole kernel (72 exps = 92us).
  - PE filler work (qT for at=1/q-chunk 1, Wo for q-chunk 0) is
    interleaved into the attention st-loop in ~0.5-1us pieces to soak up
    the PE slack under the exp pacing; gates keep not-yet-satisfiable
    pieces out of the in-order PE queue.
  - Softmax denominator via reciprocal_approx_fast (~5x faster than
    nc.vector.reciprocal; 18 bits vs bf16's 8). NOTE: custom DVE ops
    ignore the partition offset of their input AP — the denominator row
    must be copied to a partition-0 tile first.
  - Last head's normalize splits into 512-col halves so the tail Wo for
    qt8-11 unblocks after the first half; Wo qt4-7 are reserved for the
    tail to keep the PE p-state up through the norm chain.
  - Output stored bf16 (halves the 8.4MB output DMA; host partial-sums
    in f32).
  - PSUM: scores 2x[128,1024] + ctx 1x + filler 1x = exactly 8 banks
    (matmul outputs are capped at 512 f32 cols = 1 bank, so scores need
    2 matmuls per tile; wider exp tiles don't fit PSUM).
  - Device note: sustained load trips DVFS throttling (~20% on PE and
    ACT clocks); back-to-back runs measure ~180us vs ~154us cold.
"""

import sys
import numpy as np

for _p in ("/opt/trn_rl_repo",):
    if _p not in sys.path:
        sys.path.insert(0, _p)

import ml_dtypes

B, Q, S, D = 2, 2048, 2048, 1024
H, A = 16, 64
HG = 4            # heads per core
NCORES = 8
NEG = -1e9
MIN_NST = 9       # S tiles after mask pruning (1152 slots; count ~1024)

_cache = {}


def _build_program(nst):
    import concourse.bass as bass  # noqa
    import concourse.bacc as bacc
    import concourse.tile as tile
    from concourse import mybir

    f32 = mybir.dt.float32
    bf16 = mybir.dt.bfloat16
    EXP = mybir.ActivationFunctionType.Exp
    MULT = mybir.AluOpType.mult

    nc = bacc.Bacc("TRN2", target_bir_lowering=False, debug=False)

    SP = nst * 128  # padded kept-S extent
    # DMA-friendly layouts: outer piece index keeps per-partition lines
    # long (6-8KB) so the single hardware DMA queue runs at full rate,
    # while pieces still land progressively for compute overlap.
    NKP = (nst + 2) // 3          # ikt pieces of 3 s-tiles (384 cols)
    KPW = 3 * 128
    iqt = nc.dram_tensor("iqt", [128, 4, 8, 512], bf16, kind="ExternalInput").ap()
    ikt = nc.dram_tensor("ikt", [128, NKP, 8, KPW], bf16, kind="ExternalInput").ap()
    wq = nc.dram_tensor("wq", [128, 8, 256], bf16, kind="ExternalInput").ap()
    wk = nc.dram_tensor("wk", [128, 8, 256], bf16, kind="ExternalInput").ap()
    wv = nc.dram_tensor("wv", [128, 8, 256], bf16, kind="ExternalInput").ap()
    wo = nc.dram_tensor("wo", [128, 2, D], bf16, kind="ExternalInput").ap()
    mb = nc.dram_tensor("mb", [128, nst], f32, kind="ExternalInput").ap()
    out = nc.dram_tensor("out", [128, 16, D], bf16, kind="ExternalOutput").ap()

    NDT = D // 128          # 8 d tiles

    with tile.TileContext(nc) as tc:
        with (
            tc.tile_pool(name="persist", bufs=1) as persist,
            tc.tile_pool(name="expp", bufs=4) as expp,
            tc.tile_pool(name="outp", bufs=3) as outp,
            tc.tile_pool(name="srp", bufs=2) as srp,
            tc.tile_pool(name="scp", bufs=2, space="PSUM") as scp,
            tc.tile_pool(name="ctxp", bufs=1, space="PSUM") as ctxp,
            tc.tile_pool(name="fillp", bufs=1, space="PSUM") as fillp,
        ):
            # ---- persistent tiles ----
            wk_sb = persist.tile([128, 8, 256], bf16, tag="wk")
            wq_sb = persist.tile([128, 8, 256], bf16, tag="wq")
            wv_sb = persist.tile([128, 8, 256], bf16, tag="wv")
            wo_sb = persist.tile([128, 2, D], bf16, tag="wo")
            iqt_sb = persist.tile([128, 4, 8, 512], bf16, tag="iqt")
            ikt_sb = persist.tile([128, NKP, 8, KPW], bf16, tag="ikt")
            mb_sb = persist.tile([128, nst], f32, tag="mb")
            qt_sb = persist.tile([128, 2, Q], bf16, tag="qt")
            kt_sb = persist.tile([128, 2, SP], bf16, tag="kt")
            # V padded to 128 cols; col 64 = ones (softmax denominator
            # row), cols 65.. = zeros.
            v_sb = persist.tile([128, nst, HG, 128], bf16, tag="v")
            ctxn = [
                [
                    persist.tile(
                        [128, 1024], bf16, tag=f"ctxn{qc}{t}", name=f"ctxn{qc}{t}"
                    )
                    for t in range(2)
                ]
                for qc in range(2)
            ]

            # ---- DMA issue order == consumption order ----
            nc.sync.dma_start(wk_sb[:], wk[:])
            for p in range(NKP):
                nc.sync.dma_start(ikt_sb[:, p], ikt[:, p])
            nc.sync.dma_start(wv_sb[:], wv[:])
            nc.sync.dma_start(wq_sb[:], wq[:])
            nc.sync.dma_start(mb_sb[:], mb[:])
            for p in range(4):
                nc.sync.dma_start(iqt_sb[:, p], iqt[:, p])
            nc.sync.dma_start(wo_sb[:], wo[:])

            # v ones/zeros columns (runs during the DMA-bound startup)
            nc.vector.memset(v_sb[:], 0.0)
            nc.vector.memset(v_sb[:, :, :, 64:65], 1.0)

            # ---- kT projection, piece-major (pipelines with ikt DMA) ----
            for p in range(NKP):
                for at in range(2):
                    ps = scp.tile([128, 1024], f32, tag="sc", name="ps")
                    for dt_i in range(NDT):
                        nc.tensor.matmul(
                            ps[:, :KPW],
                            lhsT=wk_sb[:, dt_i, at * 128:(at + 1) * 128],
                            rhs=ikt_sb[:, p, dt_i, :],
                            start=(dt_i == 0),
                            stop=(dt_i == NDT - 1),
                        )
                    nc.vector.tensor_copy(
                        out=kt_sb[:, at, p * KPW:(p + 1) * KPW], in_=ps[:, :KPW]
                    )

            # ---- V projection: natural [s, a] per head (+ ones col).
            # All of V runs pre-attention: it overlaps the iqt DMA that
            # gates the qT projection anyway, so deferring it into h0 only
            # crowds the PE there (measured +3us).
            NVPRE = nst

            def emit_v(st, pool, tag):
                p, j = divmod(st, 3)
                ps = pool.tile([128, HG, 64], f32, tag=tag, name="ps")
                for dt_i in range(NDT):
                    nc.tensor.matmul(
                        ps[:],
                        lhsT=ikt_sb[:, p, dt_i, j * 128:(j + 1) * 128],
                        rhs=wv_sb[:, dt_i, :],
                        start=(dt_i == 0),
                        stop=(dt_i == NDT - 1),
                    )
                nc.vector.tensor_copy(out=v_sb[:, st, :, 0:64], in_=ps[:])

            for st in range(NVPRE):
                emit_v(st, scp, "sc")

            # ---- qT projection, at=0 of q-chunk 0 only (all h0/h1 needs);
            #      at=1 runs as attention filler before h2 ----
            qt0 = scp.tile([128, 1024], f32, tag="sc", name="qt0")
            for p in (0, 1):
                for dt_i in range(NDT):
                    nc.tensor.matmul(
                        qt0[:, p * 512:(p + 1) * 512],
                        lhsT=wq_sb[:, dt_i, 0:128],
                        rhs=iqt_sb[:, p, dt_i, :],
                        start=(dt_i == 0),
                        stop=(dt_i == NDT - 1),
                    )
            nc.vector.tensor_copy(out=qt_sb[:, 0, 0:1024], in_=qt0[:])

            # ---- filler generators: PE work interleaved into attention ----
            def qt_fill_gen(at, qc):
                # one (at, qc) chunk of the q projection in 2-matmul pieces,
                # with the psum->sbuf copy attached to the last piece
                ps = fillp.tile([128, 1024], f32, tag="fill", name="ps")
                for pi, p in enumerate((2 * qc, 2 * qc + 1)):
                    for d0 in range(0, NDT, 2):
                        for dt_i in (d0, d0 + 1):
                            nc.tensor.matmul(
                                ps[:, pi * 512:(pi + 1) * 512],
                                lhsT=wq_sb[:, dt_i,
                                           at * 128:(at + 1) * 128],
                                rhs=iqt_sb[:, p, dt_i, :],
                                start=(dt_i == 0),
                                stop=(dt_i == NDT - 1),
                            )
                        last = (pi == 1 and d0 == NDT - 2)
                        if last:
                            nc.vector.tensor_copy(
                                out=qt_sb[:, at, qc * 1024:(qc + 1) * 1024],
                                in_=ps[:],
                            )
                        yield

            def emit_wo_mm(qt, ps, c):
                for t in range(2):
                    nc.tensor.matmul(
                        ps[:, c:c + 512],
                        lhsT=ctxn[qt // 8][t][:, (qt % 8) * 128:
                                              (qt % 8 + 1) * 128],
                        rhs=wo_sb[:, t, c:c + 512],
                        start=(t == 0),
                        stop=(t == 1),
                    )

            def emit_wo_out(qt, ps, on_act=False):
                ob = outp.tile([128, 1024], bf16, tag="ob", name="ob")
                if on_act:
                    nc.scalar.copy(out=ob[:], in_=ps[:])
                else:
                    nc.vector.tensor_copy(out=ob[:], in_=ps[:])
                nc.sync.dma_start(out[:, qt, :], ob[:])

            # qt 4-7 are reserved for the tail so the PE has ready work to
            # chew on while the last head's norm chain serializes (idling
            # there also drops the PE p-state, slowing the tail matmuls).
            def wo_qc0_gen():
                for qt in range(4):
                    ps = fillp.tile([128, 1024], f32, tag="fill", name="ps")
                    emit_wo_mm(qt, ps, 0)
                    yield
                    emit_wo_mm(qt, ps, 512)
                    emit_wo_out(qt, ps)
                    yield

            # ---- attention: flat (qc, h, st) pipeline, sc one ahead ----
            seq = [(qc, h, st) for qc in range(2) for h in range(HG)
                   for st in range(nst)]
            n = len(seq)

            def emit_sc(qc, h, st):
                po = (h % 2) * 64
                ti = h // 2
                ps = scp.tile([128, 1024], f32, tag="sc", name="ps")
                q0 = qc * 1024
                for c in (0, 512):
                    nc.tensor.matmul(
                        ps[:, c:c + 512],
                        lhsT=kt_sb[po:po + 64, ti, st * 128:(st + 1) * 128],
                        rhs=qt_sb[po:po + 64, ti, q0 + c:q0 + c + 512],
                        start=True,
                        stop=True,
                    )
                return ps

            def v_rest_gen():
                for st in range(NVPRE, nst):
                    emit_v(st, fillp, "fill")
                    yield

            # (filler, earliest (qc,h) it may be popped at): wo_qc0 reads
            # ctxn written by the qc0-h3 norm, so popping it earlier would
            # block the in-order PE queue on an unfired semaphore.
            fillers = [
                (v_rest_gen(), (0, 0)),        # V tiles 5.. feed h0's ctx
                (qt_fill_gen(1, 0), (0, 1)),   # at=1 of qc0, before h2
                (qt_fill_gen(0, 1), (0, 2)),
                (qt_fill_gen(1, 1), (0, 2)),
                (wo_qc0_gen(), (1, 1)),
            ]
            fill_i = 0  # index into fillers; advance when exhausted

            def pop_filler(pos):
                nonlocal fill_i
                while fill_i < len(fillers):
                    gen, gate = fillers[fill_i]
                    if pos is not None and pos < gate:
                        return False
                    try:
                        next(gen)
                        return True
                    except StopIteration:
                        fill_i += 1
                return False

            sc_tiles = {}
            sc_tiles[0] = emit_sc(*seq[0])
            ctx_cur = None
            for i, (qc, h, st) in enumerate(seq):
                # one-ahead scores matmul
                if i + 1 < n:
                    sc_tiles[i + 1] = emit_sc(*seq[i + 1])
                ps = sc_tiles.pop(i)
                ex = expp.tile([128, 1024], bf16, tag="exp", name="ex")
                nc.scalar.activation(
                    out=ex[:], in_=ps[:], func=EXP,
                    bias=mb_sb[:, st:st + 1], scale=0.125,
                )
                if st == 0:
                    ctx_cur = ctxp.tile([128, 1024], f32, tag="ctx", name="ctx")
                for c in (0, 512):
                    nc.tensor.matmul(
                        ctx_cur[:, c:c + 512],
                        lhsT=v_sb[:, st, h, :],
                        rhs=ex[:, c:c + 512],
                        start=(st == 0),
                        stop=(st == nst - 1),
                    )
                # one filler piece per iteration (gates in `fillers` keep
                # not-yet-satisfiable work out of the in-order PE queue)
                pop_filler((qc, h))
                if st == nst - 1:
                    po = (h % 2) * 64
                    ti = h // 2
                    # normalize: ctx/denom -> ctxn (packed 2 heads).
                    # reciprocal_approx_fast is a custom DVE op that only
                    # reads partition 0, so the denominator row must first
                    # be copied to its own partition-0 tile.
                    den = srp.tile([1, 1024], f32, tag="den", name="den")
                    ctxu = srp.tile([65, 1024], f32, tag="ctxu", name="ctxu")
                    recip = srp.tile([1, 1024], f32, tag="recip", name="recip")
                    bcd = srp.tile([64, 1024], f32, tag="bcd", name="bcd")
                    if i == n - 1:
                        # Last head (critical path): denominator copy on the
                        # idle ACT engine, and the recip/broadcast/multiply
                        # chain split into 512-col halves — the tail Wo for
                        # qt8-11 only reads cols 0-511 of ctxn, so it
                        # unblocks after the first half-multiply.
                        nc.scalar.copy(out=den[:], in_=ctx_cur[64:65, :])
                        nc.vector.tensor_copy(
                            out=ctxu[0:64, :], in_=ctx_cur[0:64, :]
                        )
                        for c in (0, 512):
                            nc.vector.reciprocal_approx_fast(
                                recip[:, c:c + 512], den[:, c:c + 512]
                            )
                        for c in (0, 512):
                            nc.gpsimd.partition_broadcast(
                                bcd[:, c:c + 512], recip[:, c:c + 512]
                            )
                            nc.vector.tensor_tensor(
                                ctxn[qc][ti][po:po + 64, c:c + 512],
                                ctxu[0:64, c:c + 512], bcd[:, c:c + 512], MULT,
                            )
                    else:
                        # One [65,1024] copy frees the ctx PSUM bank for the
                        # next head ~1us sooner; den is then sliced in SBUF.
                        nc.vector.tensor_copy(out=ctxu[:], in_=ctx_cur[:65, :])
                        nc.vector.tensor_copy(out=den[:], in_=ctxu[64:65, :])
                        nc.vector.reciprocal_approx_fast(recip[:], den[:])
                        nc.gpsimd.partition_broadcast(bcd[:], recip[:])
                        nc.vector.tensor_tensor(
                            ctxn[qc][ti][po:po + 64, :],
                            ctxu[0:64, :], bcd[:], MULT,
                        )

            # ---- drain leftover fillers, then tail Wo ----
            # qt 4-7 (q-chunk 0) first: they depend only on qc0 ctxn, so the
            # PE runs them while the last head's norm chain completes.
            # Three-slot psum rotation (scp x2 + fillp) and output copies
            # split across the now-idle ACT and DVE keep the pace at the
            # matmul rate instead of the copy rate.
            while pop_filler(None):
                pass
            for k, qt in enumerate(range(4, 16)):
                if k % 3 == 2:
                    ps = fillp.tile([128, 1024], f32, tag="fill", name="ps")
                else:
                    ps = scp.tile([128, 1024], f32, tag="sc", name="ps")
                emit_wo_mm(qt, ps, 0)
                emit_wo_mm(qt, ps, 512)
                ob = outp.tile([128, 1024], bf16, tag="ob", name="ob")
                nc.scalar.copy(out=ob[:, 0:512], in_=ps[:, 0:512])
                nc.vector.tensor_copy(out=ob[:, 512:1024], in_=ps[:, 512:1024])
                nc.sync.dma_start(out[:, qt, :], ob[:])

    nc.compile()
    return nc


def _get_program(nst):
    if nst not in _cache:
        _cache[nst] = _build_program(nst)
    return _cache[nst]


def _prep_inputs(iQ, iK, mask, Wq, Wkv, Wo):
    """Build the 8 per-core input maps (host-side shard + prune + cast)."""
    bf = ml_dtypes.bfloat16
    iQ = np.asarray(iQ, dtype=np.float32)
    iK = np.asarray(iK, dtype=np.float32)
    mask = np.asarray(mask)
    Wq = np.asarray(Wq, dtype=np.float32)
    Wkv = np.asarray(Wkv, dtype=np.float32)
    Wo = np.asarray(Wo, dtype=np.float32)

    def tile_kxn(a):  # [K=1024, N] -> [128, K/128, N]
        K, N = a.shape
        return np.ascontiguousarray(
            a.reshape(K // 128, 128, N).transpose(1, 0, 2)
        )

    kept = [np.flatnonzero(~mask[b, 0]) for b in range(B)]
    nst = max(MIN_NST, max((len(k) + 127) // 128 for k in kept))
    nst = ((nst + 2) // 3) * 3   # kernel wants 3-s-tile DMA pieces
    SP = nst * 128

    per_b = {}
    for b in range(B):
        nk = len(kept[b])
        ikt_full = np.zeros((1024, SP), dtype=np.float32)
        ikt_full[:, :nk] = iK[b][kept[b], :].T
        bias = np.full(SP, np.float32(NEG), dtype=np.float32)
        bias[:nk] = 0.0
        # [128, 8, SP] -> piece-major [128, SP/384, 8, 384] so each DMA
        # piece is one contiguous long per-partition line
        ikt_t = tile_kxn(ikt_full).astype(bf)
        ikt_t = np.ascontiguousarray(
            ikt_t.reshape(128, 8, nst // 3, 384).transpose(0, 2, 1, 3)
        )
        iqt_t = tile_kxn(iQ[b].T).astype(bf)
        iqt_t = np.ascontiguousarray(
            iqt_t.reshape(128, 8, 4, 512).transpose(0, 2, 1, 3)
        )
        per_b[b] = {
            "iqt": iqt_t,
            "ikt": ikt_t,
            "mb": np.ascontiguousarray(bias.reshape(nst, 128).T),
        }
    in_maps = []
    for c in range(NCORES):
        b, g = divmod(c, NCORES // B)
        cols = slice(g * 256, (g + 1) * 256)
        wo_g = Wo[g * 256:(g + 1) * 256, :]          # [256, 1024]
        in_maps.append({
            "iqt": per_b[b]["iqt"],
            "ikt": per_b[b]["ikt"],
            "mb": per_b[b]["mb"],
            "wq": tile_kxn(Wq[:, cols]).astype(bf),
            "wk": tile_kxn(Wkv[:, cols]).astype(bf),
            "wv": tile_kxn(Wkv[:, 1024 + g * 256:1024 + (g + 1) * 256]).astype(bf),
            "wo": np.ascontiguousarray(
                wo_g.reshape(2, 128, D).transpose(1, 0, 2)
            ).astype(bf),
        })
    return in_maps, nst


def _run(inputs, trace=False):
    from concourse.bass_utils import run_bass_kernel_spmd

    in_maps, nst = _prep_inputs(**inputs)
    nc = _get_program(nst)
    res = run_bass_kernel_spmd(
        nc, in_maps, list(range(NCORES)), trace=trace
    )
    outs = []
    for b in range(B):
        acc = None
        for g in range(NCORES // B):
            o = np.asarray(
                res.results[b * (NCORES // B) + g]["out"], dtype=np.float32
            )
            acc = o if acc is None else acc + o
        # [128, 16, 1024] -> [2048, 1024]
        outs.append(acc.transpose(1, 0, 2).reshape(Q, D))
    return np.stack(outs), res


def kernel(**inputs):
    out, _ = _run(inputs, trace=False)
    return out
